# revision 75
# baseline (speedup 1.0000x reference)
"""BiAttention (BiDAF-style) kernel for Trainium2, 8 NeuronCores.

Reference math (T=4096, d=512):
    context  = x[0,0]; question = x[1,0]
    S[i,j]   = w1.c_i + w2.q_j + (c_i*w3).q_j
    A        = softmax_j(S)          # w1.c_i is constant per row -> cancels
    U_A      = A @ question
    b        = max_j A[i,j]
    h        = b @ context           # global over T -> one AllReduce
    G        = [context, U_A, context*U_A, context*h]

Sharding: context rows (rows of S/A/U_A/G) split across 8 cores (512 rows
each); question replicated; h all-reduced (2 KB fp16).

Per-core schedule (all big matmuls fp8e4 DoubleRow, K=256 per instruction,
0.5 cycles/row -- 4x the bf16 FLOP rate):
  S^T[j,i] is computed directly in transposed layout (j on partitions):
      S^T = qt8.T @ cw8 + qtr8.T @ cw8 + qt8.T @ cwr8
  where qt8/cw8 are fp8 quantizations of question.T and (c*w3 + w2).T and
  qtr8/cwr8 are fp8 residuals (x - fp8(x)): a 3-term compensated product
  with ~fp12 accuracy at 75% of the fp16 matmul cost (needed for the b ->
  h -> c*H_A path; pure fp8 fails the 2e-2 gate).  The w2.q_j bias rides
  inside cw8: the contraction emits it as a per-j constant.

  Per 2-jt group, pipelined: 12 DR matmuls -> exp(S^T - 3) on ACT into
  fp16 E^T (the ONLY reader of the S^T PSUM tile: a second reader would
  serialize against exp in the engine pipeline) -> DVE+Pool cast E^T to
  fp8 -> DVE fp16 running max (2x mode, two interleaved accumulators) ->
  lagged U_A(ic0) and Z matmuls (ones-column DR) so PE never waits on exp.

  Tail: zinv; max folds -> PE transpose -> strided reduce -> b = maxE*zinv;
  h partial matmul; h store -> AllReduce -> fp16 broadcast load, with the
  remaining U_A chunks, 1/Z scales, c*U_A products (all-fp16 DVE 2x) and
  G block-0..2 stores hidden under the round-trip; block 3 = c16*h_bc is
  one fused DVE op + one store.  G is staged fp16; the host upcasts.
"""

import numpy as np
import ml_dtypes

import concourse.bass as bass
import concourse.mybir as mybir
import concourse.tile as tile
from concourse import bacc
from concourse.bass_utils import run_bass_kernel_spmd
from concourse.masks import make_identity

F32 = mybir.dt.float32
F16 = mybir.dt.float16
F8 = mybir.dt.float8e4
AF = mybir.ActivationFunctionType
DR = mybir.MatmulPerfMode.DoubleRow
NP8 = ml_dtypes.float8_e4m3

T = 4096
D = 512
NCORES = 8
TL = T // NCORES          # 512 local context rows per core
P = 128
NIC = TL // P             # 4 i-chunks of 128
NJT = T // P              # 32 j-tiles of 128
NPAIR = NJT // 2          # 16 j-tile pairs (DoubleRow contraction unit)
NG = 16                   # phase-1 groups of 2 j-tiles ([128,1024] psum)
SHIFT = 3.0               # global exp shift; cancels in softmax/b


def build_kernel(collective=True, compile=True):
    nc = bacc.Bacc("TRN2", target_bir_lowering=False, debug=False,
                   num_devices=NCORES if collective else 1)

    qtt_d = nc.dram_tensor("qtt", [P, 8, T], F8, kind="ExternalInput").ap()
    qn8_d = nc.dram_tensor("qn8", [P, NJT, D], F8, kind="ExternalInput").ap()
    cwp_d = nc.dram_tensor("cwp", [P, 8, TL], F8, kind="ExternalInput").ap()
    c16_d = nc.dram_tensor("c16", [P, NIC, D], F16, kind="ExternalInput").ap()
    g_d = nc.dram_tensor("g", [TL, 4 * D], F16, kind="ExternalOutput").ap()

    with tile.TileContext(nc) as tc:
        _emit(nc, tc, qtt_d, qn8_d, cwp_d, c16_d, g_d,
              collective=collective)

    if compile:
        nc.compile()
    return nc


def _emit(nc, tc, qtt_d, qn8_d, cwp_d, c16_d, g_d,
          collective=True):
    from contextlib import ExitStack
    ctx = ExitStack()
    consts = ctx.enter_context(tc.tile_pool(name="consts", bufs=1))
    gpool = ctx.enter_context(tc.tile_pool(name="gpool", bufs=1))
    uapool = ctx.enter_context(tc.tile_pool(name="uapool", bufs=4, space="PSUM"))
    spool = ctx.enter_context(tc.tile_pool(name="spool", bufs=2, space="PSUM"))
    dram = ctx.enter_context(tc.tile_pool(name="dram", bufs=1, space="DRAM"))

    # ---- prologue: PE warm-up + constants ---------------------------------
    # Dummy matmuls keep PE busy through the HAM ramp while the first input
    # slices stream in; identity gates the (cheap) m-transposes much later.
    wa = consts.tile([P, P], F16)
    nc.vector.memset(wa, 0.0)
    wb = consts.tile([P, 512], F16)
    nc.vector.memset(wb, 0.0)
    wps = uapool.tile([P, D], F32, tag="ua", name="wps")
    for _ in range(10):
        nc.tensor.matmul(wps, lhsT=wa, rhs=wb, start=True, stop=True)

    bias_t = consts.tile([P, 1], F32)
    nc.vector.memset(bias_t, -SHIFT)
    ones8 = consts.tile([P, 2, 1], F8)
    nc.vector.memset(ones8, 1.0)
    ident32 = consts.tile([P, P], F32)
    make_identity(nc, ident32)
    # dummy exp warms the ACT table (free in the cost model, real on HW)
    warm = consts.tile([1, 1], F32)
    nc.vector.memset(warm, 0.0)
    nc.scalar.activation(out=warm, in_=warm, func=AF.Exp)

    # ---- inputs -----------------------------------------------------------
    # Order matters: the first S^T group needs cw8+cwr8+first q slices; the
    # head-to-first-matmul latency is the sum of these serialized transfers.
    cwp = consts.tile([P, 8, TL], F8)
    nc.sync.dma_start(out=cwp[:, 0:4], in_=cwp_d[:, 0:4])
    nc.scalar.dma_start(out=cwp[:, 4:8], in_=cwp_d[:, 4:8])
    cw8 = cwp[:, 0:4]
    cwr8 = cwp[:, 4:8]
    qtt = consts.tile([P, 8, T], F8)
    qt8 = qtt[:, 0:4]
    qtr8 = qtt[:, 4:8]
    qn8 = consts.tile([P, NJT, D], F8)
    c16 = consts.tile([P, NIC, D], F16)
    # Slices ordered by need-time: S^T group g needs qt/qtr j-slice ~256*g;
    # small leading slices minimize head latency, large trailing ones cut
    # the per-DMA HWDGE tax.  qn8/c16 ride the otherwise-idle SWDGE path
    # (Pool) to keep HWDGE clear for the critical qt/qtr stream.
    qsl = [(0, 512), (512, 1024), (1024, 2048), (2048, 3072), (3072, 4096)]
    qnsl = [(0, 4), (4, 8), (8, 16), (16, 24), (24, 32)]
    for s, (lo, hi) in enumerate(qsl):
        js = slice(lo, hi)
        eng = nc.sync if s % 2 == 0 else nc.scalar
        eng.dma_start(out=qtt[:, :, js], in_=qtt_d[:, :, js])
        jc = slice(qnsl[s][0], qnsl[s][1])
        (nc.scalar if s % 2 == 0 else nc.sync).dma_start(
            out=qn8[:, jc], in_=qn8_d[:, jc])

    # ---- persistent phase-1 state ----------------------------------------
    # E^T[j,i] in fp16 (exp output; the ONLY reader of the S^T PSUM tiles --
    # a second PSUM reader would serialize against exp in the engine model)
    # and in fp8 (cast by DVE+Pool) for the DoubleRow U_A/Z matmuls.
    e16 = consts.tile([P, NJT, D], F16)
    e8 = consts.tile([P, NJT, D], F8)
    pre_e = consts.tile([P, 512], F16)
    # Two fp16 running-max accumulators (even/odd groups): consecutive chain
    # ops are independent, and fp16 gets the DVE 2x mode.  max(E) feeds b
    # directly (no second exp needed).
    m_e = consts.tile([P, 1024], F16)
    nc.vector.memset(m_e, 0.0)
    m_o = consts.tile([P, 1024], F16)
    nc.vector.memset(m_o, 0.0)

    ua_ps = [None] * NIC
    ua_ps[0] = uapool.tile([P, D], F32, tag="ua", name="ua0")
    z_ps = uapool.tile([P, D], F32, tag="ua", name="z_ps")
    nz = [0]

    def emit_z(g):
        for ic in range(NIC):
            nc.tensor.matmul(z_ps[:, ic:ic + 1],
                             lhsT=e8[:, 2 * g:2 * g + 2, ic * P:(ic + 1) * P],
                             rhs=ones8,
                             start=(nz[0] == 0), stop=(nz[0] == NG * NIC - 1),
                             perf_mode=DR, skip_group_check=True)
            nz[0] += 1

    # ---- phase 1: S^T -> exp -> (chain max, U_A for ic 0/1) ---------------
    # The e8-consuming U_A matmuls are emitted with a LAG of 2 groups: PE
    # executes in order, so placing them right after their group's S^T
    # matmuls would stall PE on that group's exp every iteration.
    LAG = 2

    def emit_consumers(g):
        for ic in (0,):
            nc.tensor.matmul(ua_ps[ic],
                             lhsT=e8[:, 2 * g:2 * g + 2, ic * P:(ic + 1) * P],
                             rhs=qn8[:, 2 * g:2 * g + 2, :],
                             start=(g == 0), stop=(g == NG - 1),
                             perf_mode=DR, skip_group_check=True)

    for g in range(NG):
        st = spool.tile([P, 1024], F32, tag="s", name=f"st{g}")
        for k in range(2):
            jt = 2 * g + k
            col = slice(k * 512, (k + 1) * 512)
            first = True
            for (lhs, rhs) in ((qt8, cw8), (qtr8, cw8), (qt8, cwr8)):
                for a in range(2):
                    nc.tensor.matmul(
                        st[:, col],
                        lhsT=lhs[:, 2 * a:2 * a + 2, jt * P:(jt + 1) * P],
                        rhs=rhs[:, 2 * a:2 * a + 2, :],
                        start=first, stop=(lhs is qt8 and rhs is cwr8
                                           and a == 1),
                        perf_mode=DR)
                    first = False
        # E^T (fp16) with the global shift; pair index == group index here
        nc.scalar.activation(out=e16[:, 2 * g:2 * g + 2, :], in_=st,
                             func=AF.Exp, bias=bias_t)
        # fp16->fp8 cast for the DR matmuls: DVE takes one jt, Pool the other
        nc.vector.tensor_copy(out=e8[:, 2 * g, :], in_=e16[:, 2 * g, :])
        nc.gpsimd.tensor_copy(out=e8[:, 2 * g + 1, :],
                              in_=e16[:, 2 * g + 1, :])
        # fp16 running max over groups (E domain -- feeds b directly)
        macc = m_e if g % 2 == 0 else m_o
        nc.vector.tensor_tensor(out=macc, in0=e16[:, 2 * g:2 * g + 2, :],
                                in1=macc, op=mybir.AluOpType.max)
        if g == 6:
            nc.gpsimd.dma_start(out=c16, in_=c16_d)
        if g >= LAG:
            emit_consumers(g - LAG)
            emit_z(g - LAG)
        # partial max pre-folds once each accumulator is final
        if g == NG - 1:
            nc.vector.tensor_tensor(out=pre_e[:, 0:512],
                                    in0=m_e[:, 512:], in1=m_e[:, :512],
                                    op=mybir.AluOpType.max)
    for g in range(NG - LAG, NG):
        emit_consumers(g)
        emit_z(g)

    # ---- phase 2 ----------------------------------------------------------
    # Z finished during phase 1 (lagged); here: zinv, fold -> transpose ->
    # b16 -> h launch, then ALL remaining U_A under the h DMA round-trip.
    # The whole b -> h -> round-trip chain is emitted at high priority so
    # the scheduler never parks its DMAs behind the (slack-rich) G stores.
    with tc.high_priority():
        zinv = consts.tile([P, NIC], F32)
        nc.vector.reciprocal(out=zinv, in_=z_ps[:, 0:NIC])

        nc.vector.tensor_tensor(out=pre_e[:, 0:512], in0=m_o[:, 512:],
                                in1=pre_e[:, 0:512], op=mybir.AluOpType.max)
        mf = consts.tile([P, 512], F32)
        nc.vector.tensor_tensor(out=mf, in0=m_o[:, :512],
                                in1=pre_e[:, 0:512],
                                op=mybir.AluOpType.max)
        tp = uapool.tile([P, 512], F32, tag="ua", name="tp_m")
        for ic in range(NIC):
            nc.tensor.transpose(tp[:, ic * P:(ic + 1) * P],
                                mf[:, ic * P:(ic + 1) * P], ident32)
        emax = consts.tile([P, NIC], F32)
        nc.vector.tensor_reduce(out=emax,
                                in_=tp.rearrange("p (ic q) -> p ic q", q=P),
                                axis=mybir.AxisListType.X,
                                op=mybir.AluOpType.max)
        b16 = consts.tile([P, NIC], F16)
        nc.vector.tensor_tensor(out=b16, in0=emax, in1=zinv,
                                op=mybir.AluOpType.mult)

        # h partial; launch the DMA round-trip
        h_ps = uapool.tile([P, D], F32, tag="ua", name="h_ps")
        for ic in range(NIC):
            for dc in range(4):
                nc.tensor.matmul(h_ps[:, dc:dc + 1],
                                 lhsT=c16[:, ic, dc * P:(dc + 1) * P],
                                 rhs=b16[:, ic:ic + 1],
                                 start=(ic == 0 and dc == 0),
                                 stop=(ic == NIC - 1 and dc == 3),
                                 skip_group_check=True)
        h_sb = consts.tile([P, 4], F16)
        nc.vector.tensor_copy(out=h_sb, in_=h_ps[:, 0:4])
        hp_dram = dram.tile([D], F16)
        hs_dram = dram.tile([D], F16)
        hp_ap = hp_dram[:]
        nc.sync.dma_start(out=hp_ap.rearrange("(dc p) -> p dc", p=P),
                          in_=h_sb)
        if collective:
            nc.gpsimd.collective_compute(
                "AllReduce", mybir.AluOpType.add,
                replica_groups=[list(range(NCORES))],
                ins=[hp_dram.opt()], outs=[hs_dram.opt()],
            )
        else:
            nc.sync.dma_start(out=hs_dram[:], in_=hp_dram[:])
        hs_ap = hs_dram[:]
        h_bc = consts.tile([P, D], F16)
        nc.sync.dma_start(
            out=h_bc,
            in_=bass.AP(tensor=hs_ap.tensor, offset=hs_ap.offset,
                        ap=[[0, P], [1, D]]),
        )

    def emit_ua_phase2(ic):
        for pair in range(NPAIR):
            nc.tensor.matmul(ua_ps[ic],
                             lhsT=e8[:, 2 * pair:2 * pair + 2,
                                     ic * P:(ic + 1) * P],
                             rhs=qn8[:, 2 * pair:2 * pair + 2, :],
                             start=(pair == 0), stop=(pair == NPAIR - 1),
                             perf_mode=DR, skip_group_check=True)

    for ic in (1, 2, 3):
        ua_ps[ic] = uapool.tile([P, D], F32, tag="ua", name=f"ua{ic}")
        emit_ua_phase2(ic)

    # ---- G assembly (one fp16 staging tile) + 2 stores --------------------
    # gst[p, ic, :] holds G row ic*128+p.  Blocks: 0=c, 1=U_A, 2=c*U_A,
    # 3=c*h.  cu/ch are all-fp16 DVE products (2x mode); block 3 waits only
    # on the h broadcast.
    gst = gpool.tile([P, NIC, 4 * D], F16)
    nc.vector.tensor_copy(out=gst[:, :, 0:D], in_=c16)
    for ic in range(NIC):
        nc.scalar.activation(out=gst[:, ic, D:2 * D], in_=ua_ps[ic],
                             func=AF.Copy, scale=zinv[:, ic:ic + 1])
        nc.vector.tensor_tensor(out=gst[:, ic, 2 * D:3 * D],
                                in0=c16[:, ic, :], in1=gst[:, ic, D:2 * D],
                                op=mybir.AluOpType.mult)
        nc.scalar.dma_start(out=g_d[ic * P:(ic + 1) * P, 0:3 * D],
                             in_=gst[:, ic, 0:3 * D])
    with tc.high_priority():
        h_bc4 = bass.AP(tensor=h_bc.tensor, offset=h_bc.offset,
                        ap=[h_bc.ap[0], [0, NIC], h_bc.ap[1]])
        nc.vector.tensor_tensor(out=gst[:, :, 3 * D:4 * D], in0=c16,
                                in1=h_bc4, op=mybir.AluOpType.mult)
        nc.scalar.dma_start(
            out=g_d.rearrange("(ic p) c -> p ic c", p=P)[:, :, 3 * D:4 * D],
            in_=gst[:, :, 3 * D:4 * D])

    ctx.close()


# ---------------------------------------------------------------------------


def _prep_inputs(x, w):
    """Host-side quantization + layout. Returns per-core in_maps."""
    context = np.ascontiguousarray(x[0, 0]).astype(np.float32)   # (T, D)
    question = np.ascontiguousarray(x[1, 0]).astype(np.float32)  # (T, D)
    w = np.asarray(w, dtype=np.float32)
    w2 = w[D:2 * D]
    w3 = w[2 * D:3 * D]

    # question.T in [p, dc, j] layout, fp8 + fp8 residual
    qT = question.T.reshape(4, P, T)                  # [dc, p, j]
    qT = np.ascontiguousarray(qT.transpose(1, 0, 2))  # [p, dc, j]
    qt8 = qT.astype(NP8)
    qtr8 = (qT - qt8.astype(np.float32)).astype(NP8)
    qtt = np.concatenate([qt8, qtr8], axis=1)         # [p, 8, j]

    # question natural in [p, jc, d] layout, fp8
    qn = question.reshape(NJT, P, D)                  # [jc, p, d]
    qn8 = np.ascontiguousarray(qn.transpose(1, 0, 2)).astype(NP8)

    cw_full = context * w3[None, :] + w2[None, :]     # (T, D)

    in_maps = []
    for core in range(NCORES):
        rows = slice(core * TL, (core + 1) * TL)
        cw = cw_full[rows]                            # (TL, D)
        cwT = cw.T.reshape(4, P, TL)                  # [dc, p, i]
        cwT = np.ascontiguousarray(cwT.transpose(1, 0, 2))
        cw8 = cwT.astype(NP8)
        cwr8 = (cwT - cw8.astype(np.float32)).astype(NP8)
        cwp = np.concatenate([cw8, cwr8], axis=1)     # [p, 8, i]
        cn = context[rows].reshape(NIC, P, D)         # [ic, p, d]
        c16 = np.ascontiguousarray(cn.transpose(1, 0, 2)).astype(np.float16)
        in_maps.append({
            "qtt": qtt, "qn8": qn8, "cwp": cwp, "c16": c16,
        })
    return in_maps


_NC_CACHE = {}


def _get_nc():
    if "nc" not in _NC_CACHE:
        _NC_CACHE["nc"] = build_kernel()
    return _NC_CACHE["nc"]


def kernel(x: np.ndarray, kernel: np.ndarray) -> np.ndarray:
    nc = _get_nc()
    in_maps = _prep_inputs(x, kernel)
    res = run_bass_kernel_spmd(nc, in_maps, core_ids=list(range(NCORES)))
    g = np.concatenate([res.results[core]["g"] for core in range(NCORES)],
                       axis=0)
    return g.astype(np.float32)


# revision 76
# speedup vs baseline: 1.0030x; 1.0030x over previous
"""BiAttention (BiDAF-style) kernel for Trainium2, 8 NeuronCores.

Reference math (T=4096, d=512):
    context  = x[0,0]; question = x[1,0]
    S[i,j]   = w1.c_i + w2.q_j + (c_i*w3).q_j
    A        = softmax_j(S)          # w1.c_i is constant per row -> cancels
    U_A      = A @ question
    b        = max_j A[i,j]
    h        = b @ context           # global over T -> one AllReduce
    G        = [context, U_A, context*U_A, context*h]

Sharding: context rows (rows of S/A/U_A/G) split across 8 cores (512 rows
each); question replicated; h all-reduced (2 KB fp16).

Per-core schedule (all big matmuls fp8e4 DoubleRow, K=256 per instruction,
0.5 cycles/row -- 4x the bf16 FLOP rate):
  S^T[j,i] is computed directly in transposed layout (j on partitions):
      S^T = qt8.T @ cw8 + qtr8.T @ cw8 + qt8.T @ cwr8
  where qt8/cw8 are fp8 quantizations of question.T and (c*w3 + w2).T and
  qtr8/cwr8 are fp8 residuals (x - fp8(x)): a 3-term compensated product
  with ~fp12 accuracy at 75% of the fp16 matmul cost (needed for the b ->
  h -> c*H_A path; pure fp8 fails the 2e-2 gate).  The w2.q_j bias rides
  inside cw8: the contraction emits it as a per-j constant.

  Per 2-jt group, pipelined: 12 DR matmuls -> exp(S^T - 3) on ACT into
  fp16 E^T (the ONLY reader of the S^T PSUM tile: a second reader would
  serialize against exp in the engine pipeline) -> DVE+Pool cast E^T to
  fp8 -> DVE fp16 running max (2x mode, two interleaved accumulators) ->
  lagged U_A(ic0) and Z matmuls (ones-column DR) so PE never waits on exp.

  Tail: zinv; max folds -> PE transpose -> strided reduce -> b = maxE*zinv;
  h partial matmul; h store -> AllReduce -> fp16 broadcast load, with the
  remaining U_A chunks, 1/Z scales, c*U_A products (all-fp16 DVE 2x) and
  G block-0..2 stores hidden under the round-trip; block 3 = c16*h_bc is
  one fused DVE op + one store.  G is staged fp16; the host upcasts.
"""

import numpy as np
import ml_dtypes

import concourse.bass as bass
import concourse.mybir as mybir
import concourse.tile as tile
from concourse import bacc
from concourse.bass_utils import run_bass_kernel_spmd
from concourse.masks import make_identity

F32 = mybir.dt.float32
F16 = mybir.dt.float16
F8 = mybir.dt.float8e4
AF = mybir.ActivationFunctionType
DR = mybir.MatmulPerfMode.DoubleRow
NP8 = ml_dtypes.float8_e4m3

T = 4096
D = 512
NCORES = 8
TL = T // NCORES          # 512 local context rows per core
P = 128
NIC = TL // P             # 4 i-chunks of 128
NJT = T // P              # 32 j-tiles of 128
NPAIR = NJT // 2          # 16 j-tile pairs (DoubleRow contraction unit)
NG = 16                   # phase-1 groups of 2 j-tiles ([128,1024] psum)
SHIFT = 3.0               # global exp shift; cancels in softmax/b


def build_kernel(collective=True, compile=True):
    nc = bacc.Bacc("TRN2", target_bir_lowering=False, debug=False,
                   num_devices=NCORES if collective else 1)

    qtt_d = nc.dram_tensor("qtt", [P, 8, T], F8, kind="ExternalInput").ap()
    qn8_d = nc.dram_tensor("qn8", [P, NJT, D], F8, kind="ExternalInput").ap()
    cwp_d = nc.dram_tensor("cwp", [P, 8, TL], F8, kind="ExternalInput").ap()
    c16_d = nc.dram_tensor("c16", [P, NIC, D], F16, kind="ExternalInput").ap()
    g_d = nc.dram_tensor("g", [TL, 4 * D], F16, kind="ExternalOutput").ap()

    with tile.TileContext(nc) as tc:
        _emit(nc, tc, qtt_d, qn8_d, cwp_d, c16_d, g_d,
              collective=collective)

    if compile:
        nc.compile()
    return nc


def _emit(nc, tc, qtt_d, qn8_d, cwp_d, c16_d, g_d,
          collective=True):
    from contextlib import ExitStack
    ctx = ExitStack()
    consts = ctx.enter_context(tc.tile_pool(name="consts", bufs=1))
    gpool = ctx.enter_context(tc.tile_pool(name="gpool", bufs=1))
    uapool = ctx.enter_context(tc.tile_pool(name="uapool", bufs=4, space="PSUM"))
    spool = ctx.enter_context(tc.tile_pool(name="spool", bufs=2, space="PSUM"))
    dram = ctx.enter_context(tc.tile_pool(name="dram", bufs=1, space="DRAM"))

    # ---- prologue: PE warm-up + constants ---------------------------------
    # Dummy matmuls keep PE busy through the HAM ramp while the first input
    # slices stream in; identity gates the (cheap) m-transposes much later.
    wa = consts.tile([P, P], F16)
    nc.vector.memset(wa, 0.0)
    wb = consts.tile([P, 512], F16)
    nc.vector.memset(wb, 0.0)
    wps = uapool.tile([P, D], F32, tag="ua", name="wps")
    for _ in range(10):
        nc.tensor.matmul(wps, lhsT=wa, rhs=wb, start=True, stop=True)

    bias_t = consts.tile([P, 1], F32)
    nc.vector.memset(bias_t, -SHIFT)
    ones8 = consts.tile([P, 2, 1], F8)
    nc.vector.memset(ones8, 1.0)
    ident32 = consts.tile([P, P], F32)
    make_identity(nc, ident32)
    # dummy exp warms the ACT table (free in the cost model, real on HW)
    warm = consts.tile([1, 1], F32)
    nc.vector.memset(warm, 0.0)
    nc.scalar.activation(out=warm, in_=warm, func=AF.Exp)

    # ---- inputs -----------------------------------------------------------
    # Order matters: the first S^T group needs cw8+cwr8+first q slices; the
    # head-to-first-matmul latency is the sum of these serialized transfers.
    cwp = consts.tile([P, 8, TL], F8)
    nc.sync.dma_start(out=cwp[:, 0:4], in_=cwp_d[:, 0:4])
    nc.scalar.dma_start(out=cwp[:, 4:8], in_=cwp_d[:, 4:8])
    cw8 = cwp[:, 0:4]
    cwr8 = cwp[:, 4:8]
    qtt = consts.tile([P, 8, T], F8)
    qt8 = qtt[:, 0:4]
    qtr8 = qtt[:, 4:8]
    qn8 = consts.tile([P, NJT, D], F8)
    c16 = consts.tile([P, NIC, D], F16)
    # Slices ordered by need-time: S^T group g needs qt/qtr j-slice ~256*g;
    # small leading slices minimize head latency, large trailing ones cut
    # the per-DMA HWDGE tax.  qn8/c16 ride the otherwise-idle SWDGE path
    # (Pool) to keep HWDGE clear for the critical qt/qtr stream.
    qsl = [(0, 512), (512, 1024), (1024, 2048), (2048, 3072), (3072, 4096)]
    qnsl = [(0, 4), (4, 8), (8, 16), (16, 24), (24, 32)]
    for s, (lo, hi) in enumerate(qsl):
        js = slice(lo, hi)
        eng = nc.sync if s % 2 == 0 else nc.scalar
        eng.dma_start(out=qtt[:, :, js], in_=qtt_d[:, :, js])
        jc = slice(qnsl[s][0], qnsl[s][1])
        (nc.scalar if s % 2 == 0 else nc.sync).dma_start(
            out=qn8[:, jc], in_=qn8_d[:, jc])

    # ---- persistent phase-1 state ----------------------------------------
    # E^T[j,i] in fp16 (exp output; the ONLY reader of the S^T PSUM tiles --
    # a second PSUM reader would serialize against exp in the engine model)
    # and in fp8 (cast by DVE+Pool) for the DoubleRow U_A/Z matmuls.
    e16 = consts.tile([P, NJT, D], F16)
    e8 = consts.tile([P, NJT, D], F8)
    pre_e = consts.tile([P, 512], F16)
    # Two fp16 running-max accumulators (even/odd groups): consecutive chain
    # ops are independent, and fp16 gets the DVE 2x mode.  max(E) feeds b
    # directly (no second exp needed).
    m_e = consts.tile([P, 1024], F16)
    nc.vector.memset(m_e, 0.0)
    m_o = consts.tile([P, 1024], F16)
    nc.vector.memset(m_o, 0.0)

    ua_ps = [None] * NIC
    ua_ps[0] = uapool.tile([P, D], F32, tag="ua", name="ua0")
    z_ps = uapool.tile([P, D], F32, tag="ua", name="z_ps")
    nz = [0]

    def emit_z(g):
        for ic in range(NIC):
            nc.tensor.matmul(z_ps[:, ic:ic + 1],
                             lhsT=e8[:, 2 * g:2 * g + 2, ic * P:(ic + 1) * P],
                             rhs=ones8,
                             start=(nz[0] == 0), stop=(nz[0] == NG * NIC - 1),
                             perf_mode=DR, skip_group_check=True)
            nz[0] += 1

    # ---- phase 1: S^T -> exp -> (chain max, U_A for ic 0/1) ---------------
    # The e8-consuming U_A matmuls are emitted with a LAG of 2 groups: PE
    # executes in order, so placing them right after their group's S^T
    # matmuls would stall PE on that group's exp every iteration.
    LAG = 2

    def emit_consumers(g):
        for ic in (0,):
            nc.tensor.matmul(ua_ps[ic],
                             lhsT=e8[:, 2 * g:2 * g + 2, ic * P:(ic + 1) * P],
                             rhs=qn8[:, 2 * g:2 * g + 2, :],
                             start=(g == 0), stop=(g == NG - 1),
                             perf_mode=DR, skip_group_check=True)

    for g in range(NG):
        st = spool.tile([P, 1024], F32, tag="s", name=f"st{g}")
        for k in range(2):
            jt = 2 * g + k
            col = slice(k * 512, (k + 1) * 512)
            first = True
            for (lhs, rhs) in ((qt8, cw8), (qtr8, cw8), (qt8, cwr8)):
                for a in range(2):
                    nc.tensor.matmul(
                        st[:, col],
                        lhsT=lhs[:, 2 * a:2 * a + 2, jt * P:(jt + 1) * P],
                        rhs=rhs[:, 2 * a:2 * a + 2, :],
                        start=first, stop=(lhs is qt8 and rhs is cwr8
                                           and a == 1),
                        perf_mode=DR)
                    first = False
        # E^T (fp16) with the global shift; pair index == group index here
        nc.scalar.activation(out=e16[:, 2 * g:2 * g + 2, :], in_=st,
                             func=AF.Exp, bias=bias_t)
        # fp16->fp8 cast for the DR matmuls: DVE takes one jt, Pool the other
        nc.vector.tensor_copy(out=e8[:, 2 * g, :], in_=e16[:, 2 * g, :])
        nc.gpsimd.tensor_copy(out=e8[:, 2 * g + 1, :],
                              in_=e16[:, 2 * g + 1, :])
        # fp16 running max over groups (E domain -- feeds b directly)
        macc = m_e if g % 2 == 0 else m_o
        nc.vector.tensor_tensor(out=macc, in0=e16[:, 2 * g:2 * g + 2, :],
                                in1=macc, op=mybir.AluOpType.max)
        if g == 6:
            nc.gpsimd.dma_start(out=c16, in_=c16_d)
        if g >= LAG:
            emit_consumers(g - LAG)
            emit_z(g - LAG)
        # partial max pre-folds once each accumulator is final
        if g == NG - 1:
            nc.vector.tensor_tensor(out=pre_e[:, 0:512],
                                    in0=m_e[:, 512:], in1=m_e[:, :512],
                                    op=mybir.AluOpType.max)
    for g in range(NG - LAG, NG):
        emit_consumers(g)
        emit_z(g)

    # ---- phase 2 ----------------------------------------------------------
    # Z finished during phase 1 (lagged); here: zinv, fold -> transpose ->
    # b16 -> h launch, then ALL remaining U_A under the h DMA round-trip.
    # The whole b -> h -> round-trip chain is emitted at high priority so
    # the scheduler never parks its DMAs behind the (slack-rich) G stores.
    with tc.high_priority():
        zinv = consts.tile([P, NIC], F32)
        nc.vector.reciprocal(out=zinv, in_=z_ps[:, 0:NIC])

        nc.vector.tensor_tensor(out=pre_e[:, 0:512], in0=m_o[:, 512:],
                                in1=pre_e[:, 0:512], op=mybir.AluOpType.max)
        mf = consts.tile([P, 512], F32)
        nc.vector.tensor_tensor(out=mf, in0=m_o[:, :512],
                                in1=pre_e[:, 0:512],
                                op=mybir.AluOpType.max)
        tp = uapool.tile([P, 512], F32, tag="ua", name="tp_m")
        for ic in range(NIC):
            nc.tensor.transpose(tp[:, ic * P:(ic + 1) * P],
                                mf[:, ic * P:(ic + 1) * P], ident32)
        emax = consts.tile([P, NIC], F32)
        nc.vector.tensor_reduce(out=emax,
                                in_=tp.rearrange("p (ic q) -> p ic q", q=P),
                                axis=mybir.AxisListType.X,
                                op=mybir.AluOpType.max)
        b16 = consts.tile([P, NIC], F16)
        nc.vector.tensor_tensor(out=b16, in0=emax, in1=zinv,
                                op=mybir.AluOpType.mult)

        # h partial; launch the DMA round-trip
        h_ps = uapool.tile([P, D], F32, tag="ua", name="h_ps")
        for ic in range(NIC):
            for dc in range(4):
                nc.tensor.matmul(h_ps[:, dc:dc + 1],
                                 lhsT=c16[:, ic, dc * P:(dc + 1) * P],
                                 rhs=b16[:, ic:ic + 1],
                                 start=(ic == 0 and dc == 0),
                                 stop=(ic == NIC - 1 and dc == 3),
                                 skip_group_check=True)
        h_sb = consts.tile([P, 4], F16)
        nc.vector.tensor_copy(out=h_sb, in_=h_ps[:, 0:4])
        hp_dram = dram.tile([D], F16)
        hs_dram = dram.tile([D], F16)
        hp_ap = hp_dram[:]
        nc.sync.dma_start(out=hp_ap.rearrange("(dc p) -> p dc", p=P),
                          in_=h_sb)
        if collective:
            nc.gpsimd.collective_compute(
                "AllReduce", mybir.AluOpType.add,
                replica_groups=[list(range(NCORES))],
                ins=[hp_dram.opt()], outs=[hs_dram.opt()],
            )
        else:
            nc.sync.dma_start(out=hs_dram[:], in_=hp_dram[:])
        hs_ap = hs_dram[:]
        h_bc = consts.tile([P, D], F16)
        nc.sync.dma_start(
            out=h_bc,
            in_=bass.AP(tensor=hs_ap.tensor, offset=hs_ap.offset,
                        ap=[[0, P], [1, D]]),
        )

    def emit_ua_phase2(ic):
        for pair in range(NPAIR):
            nc.tensor.matmul(ua_ps[ic],
                             lhsT=e8[:, 2 * pair:2 * pair + 2,
                                     ic * P:(ic + 1) * P],
                             rhs=qn8[:, 2 * pair:2 * pair + 2, :],
                             start=(pair == 0), stop=(pair == NPAIR - 1),
                             perf_mode=DR, skip_group_check=True)

    for ic in (1, 2, 3):
        ua_ps[ic] = uapool.tile([P, D], F32, tag="ua", name=f"ua{ic}")
        emit_ua_phase2(ic)

    # ---- G assembly (one fp16 staging tile) + 2 stores --------------------
    # gst[p, ic, :] holds G row ic*128+p.  Blocks: 0=c, 1=U_A, 2=c*U_A,
    # 3=c*h.  cu/ch are all-fp16 DVE products (2x mode); block 3 waits only
    # on the h broadcast.
    gst = gpool.tile([P, NIC, 4 * D], F16)
    nc.vector.tensor_copy(out=gst[:, :, 0:D], in_=c16)
    for ic in range(NIC):
        nc.scalar.activation(out=gst[:, ic, D:2 * D], in_=ua_ps[ic],
                             func=AF.Copy, scale=zinv[:, ic:ic + 1])
        nc.vector.tensor_tensor(out=gst[:, ic, 2 * D:3 * D],
                                in0=c16[:, ic, :], in1=gst[:, ic, D:2 * D],
                                op=mybir.AluOpType.mult)
        nc.scalar.dma_start(out=g_d[ic * P:(ic + 1) * P, 0:3 * D],
                             in_=gst[:, ic, 0:3 * D])
    with tc.high_priority():
        h_bc4 = bass.AP(tensor=h_bc.tensor, offset=h_bc.offset,
                        ap=[h_bc.ap[0], [0, NIC], h_bc.ap[1]])
        nc.vector.tensor_tensor(out=gst[:, :, 3 * D:4 * D], in0=c16,
                                in1=h_bc4, op=mybir.AluOpType.mult)
        nc.sync.dma_start(
            out=g_d.rearrange("(ic p) c -> p ic c", p=P)[:, :, 3 * D:4 * D],
            in_=gst[:, :, 3 * D:4 * D])

    ctx.close()


# ---------------------------------------------------------------------------


def _prep_inputs(x, w):
    """Host-side quantization + layout. Returns per-core in_maps."""
    context = np.ascontiguousarray(x[0, 0]).astype(np.float32)   # (T, D)
    question = np.ascontiguousarray(x[1, 0]).astype(np.float32)  # (T, D)
    w = np.asarray(w, dtype=np.float32)
    w2 = w[D:2 * D]
    w3 = w[2 * D:3 * D]

    # question.T in [p, dc, j] layout, fp8 + fp8 residual
    qT = question.T.reshape(4, P, T)                  # [dc, p, j]
    qT = np.ascontiguousarray(qT.transpose(1, 0, 2))  # [p, dc, j]
    qt8 = qT.astype(NP8)
    qtr8 = (qT - qt8.astype(np.float32)).astype(NP8)
    qtt = np.concatenate([qt8, qtr8], axis=1)         # [p, 8, j]

    # question natural in [p, jc, d] layout, fp8
    qn = question.reshape(NJT, P, D)                  # [jc, p, d]
    qn8 = np.ascontiguousarray(qn.transpose(1, 0, 2)).astype(NP8)

    cw_full = context * w3[None, :] + w2[None, :]     # (T, D)

    in_maps = []
    for core in range(NCORES):
        rows = slice(core * TL, (core + 1) * TL)
        cw = cw_full[rows]                            # (TL, D)
        cwT = cw.T.reshape(4, P, TL)                  # [dc, p, i]
        cwT = np.ascontiguousarray(cwT.transpose(1, 0, 2))
        cw8 = cwT.astype(NP8)
        cwr8 = (cwT - cw8.astype(np.float32)).astype(NP8)
        cwp = np.concatenate([cw8, cwr8], axis=1)     # [p, 8, i]
        cn = context[rows].reshape(NIC, P, D)         # [ic, p, d]
        c16 = np.ascontiguousarray(cn.transpose(1, 0, 2)).astype(np.float16)
        in_maps.append({
            "qtt": qtt, "qn8": qn8, "cwp": cwp, "c16": c16,
        })
    return in_maps


_NC_CACHE = {}


def _get_nc():
    if "nc" not in _NC_CACHE:
        _NC_CACHE["nc"] = build_kernel()
    return _NC_CACHE["nc"]


def kernel(x: np.ndarray, kernel: np.ndarray) -> np.ndarray:
    nc = _get_nc()
    in_maps = _prep_inputs(x, kernel)
    res = run_bass_kernel_spmd(nc, in_maps, core_ids=list(range(NCORES)))
    g = np.concatenate([res.results[core]["g"] for core in range(NCORES)],
                       axis=0)
    return g.astype(np.float32)


# revision 77
# speedup vs baseline: 1.0071x; 1.0041x over previous
"""BiAttention (BiDAF-style) kernel for Trainium2, 8 NeuronCores.

Reference math (T=4096, d=512):
    context  = x[0,0]; question = x[1,0]
    S[i,j]   = w1.c_i + w2.q_j + (c_i*w3).q_j
    A        = softmax_j(S)          # w1.c_i is constant per row -> cancels
    U_A      = A @ question
    b        = max_j A[i,j]
    h        = b @ context           # global over T -> one AllReduce
    G        = [context, U_A, context*U_A, context*h]

Sharding: context rows (rows of S/A/U_A/G) split across 8 cores (512 rows
each); question replicated; h all-reduced (2 KB fp16).

Per-core schedule (all big matmuls fp8e4 DoubleRow, K=256 per instruction,
0.5 cycles/row -- 4x the bf16 FLOP rate):
  S^T[j,i] is computed directly in transposed layout (j on partitions):
      S^T = qt8.T @ cw8 + qtr8.T @ cw8 + qt8.T @ cwr8
  where qt8/cw8 are fp8 quantizations of question.T and (c*w3 + w2).T and
  qtr8/cwr8 are fp8 residuals (x - fp8(x)): a 3-term compensated product
  with ~fp12 accuracy at 75% of the fp16 matmul cost (needed for the b ->
  h -> c*H_A path; pure fp8 fails the 2e-2 gate).  The w2.q_j bias rides
  inside cw8: the contraction emits it as a per-j constant.

  Per 2-jt group, pipelined: 12 DR matmuls -> exp(S^T - 3) on ACT into
  fp16 E^T (the ONLY reader of the S^T PSUM tile: a second reader would
  serialize against exp in the engine pipeline) -> DVE+Pool cast E^T to
  fp8 -> DVE fp16 running max (2x mode, two interleaved accumulators) ->
  lagged U_A(ic0) and Z matmuls (ones-column DR) so PE never waits on exp.

  Tail: zinv; max folds -> PE transpose -> strided reduce -> b = maxE*zinv;
  h partial matmul; h store -> AllReduce -> fp16 broadcast load, with the
  remaining U_A chunks, 1/Z scales, c*U_A products (all-fp16 DVE 2x) and
  G block-0..2 stores hidden under the round-trip; block 3 = c16*h_bc is
  one fused DVE op + one store.  G is staged fp16; the host upcasts.
"""

import numpy as np
import ml_dtypes

import concourse.bass as bass
import concourse.mybir as mybir
import concourse.tile as tile
from concourse import bacc
from concourse.bass_utils import run_bass_kernel_spmd
from concourse.masks import make_identity

F32 = mybir.dt.float32
F16 = mybir.dt.float16
F8 = mybir.dt.float8e4
AF = mybir.ActivationFunctionType
DR = mybir.MatmulPerfMode.DoubleRow
NP8 = ml_dtypes.float8_e4m3

T = 4096
D = 512
NCORES = 8
TL = T // NCORES          # 512 local context rows per core
P = 128
NIC = TL // P             # 4 i-chunks of 128
NJT = T // P              # 32 j-tiles of 128
NPAIR = NJT // 2          # 16 j-tile pairs (DoubleRow contraction unit)
NG = 16                   # phase-1 groups of 2 j-tiles ([128,1024] psum)
SHIFT = 3.0               # global exp shift; cancels in softmax/b


def build_kernel(collective=True, compile=True):
    nc = bacc.Bacc("TRN2", target_bir_lowering=False, debug=False,
                   num_devices=NCORES if collective else 1)

    qtt_d = nc.dram_tensor("qtt", [P, 8, T], F8, kind="ExternalInput").ap()
    qn8_d = nc.dram_tensor("qn8", [P, NJT, D], F8, kind="ExternalInput").ap()
    cwp_d = nc.dram_tensor("cwp", [P, 8, TL], F8, kind="ExternalInput").ap()
    c16_d = nc.dram_tensor("c16", [P, NIC, D], F16, kind="ExternalInput").ap()
    g_d = nc.dram_tensor("g", [TL, 4 * D], F16, kind="ExternalOutput").ap()

    with tile.TileContext(nc) as tc:
        _emit(nc, tc, qtt_d, qn8_d, cwp_d, c16_d, g_d,
              collective=collective)

    if compile:
        nc.compile()
    return nc


def _emit(nc, tc, qtt_d, qn8_d, cwp_d, c16_d, g_d,
          collective=True):
    from contextlib import ExitStack
    ctx = ExitStack()
    consts = ctx.enter_context(tc.tile_pool(name="consts", bufs=1))
    gpool = ctx.enter_context(tc.tile_pool(name="gpool", bufs=1))
    uapool = ctx.enter_context(tc.tile_pool(name="uapool", bufs=4, space="PSUM"))
    spool = ctx.enter_context(tc.tile_pool(name="spool", bufs=2, space="PSUM"))
    dram = ctx.enter_context(tc.tile_pool(name="dram", bufs=1, space="DRAM"))

    # ---- prologue: PE warm-up + constants ---------------------------------
    # Dummy matmuls keep PE busy through the HAM ramp while the first input
    # slices stream in; identity gates the (cheap) m-transposes much later.
    wa = consts.tile([P, P], F16)
    nc.vector.memset(wa, 0.0)
    wb = consts.tile([P, 512], F16)
    nc.vector.memset(wb, 0.0)
    wps = uapool.tile([P, D], F32, tag="ua", name="wps")
    for _ in range(10):
        nc.tensor.matmul(wps, lhsT=wa, rhs=wb, start=True, stop=True)

    bias_t = consts.tile([P, 1], F32)
    nc.vector.memset(bias_t, -SHIFT)
    ones8 = consts.tile([P, 2, 1], F8)
    nc.vector.memset(ones8, 1.0)
    ident32 = consts.tile([P, P], F32)
    make_identity(nc, ident32)
    # dummy exp warms the ACT table (free in the cost model, real on HW)
    warm = consts.tile([1, 1], F32)
    nc.vector.memset(warm, 0.0)
    nc.scalar.activation(out=warm, in_=warm, func=AF.Exp)

    # ---- inputs -----------------------------------------------------------
    # Order matters: the first S^T group needs cw8+cwr8+first q slices; the
    # head-to-first-matmul latency is the sum of these serialized transfers.
    cwp = consts.tile([P, 8, TL], F8)
    nc.sync.dma_start(out=cwp[:, 0:4], in_=cwp_d[:, 0:4])
    nc.scalar.dma_start(out=cwp[:, 4:8], in_=cwp_d[:, 4:8])
    cw8 = cwp[:, 0:4]
    cwr8 = cwp[:, 4:8]
    qtt = consts.tile([P, 8, T], F8)
    qt8 = qtt[:, 0:4]
    qtr8 = qtt[:, 4:8]
    qn8 = consts.tile([P, NJT, D], F8)
    c16 = consts.tile([P, NIC, D], F16)
    # Slices ordered by need-time: S^T group g needs qt/qtr j-slice ~256*g;
    # small leading slices minimize head latency, large trailing ones cut
    # the per-DMA HWDGE tax.  qn8/c16 ride the otherwise-idle SWDGE path
    # (Pool) to keep HWDGE clear for the critical qt/qtr stream.
    qsl = [(0, 512), (512, 1024), (1024, 2048), (2048, 3072), (3072, 4096)]
    qnsl = [(0, 4), (4, 8), (8, 16), (16, 24), (24, 32)]
    for s, (lo, hi) in enumerate(qsl):
        js = slice(lo, hi)
        eng = nc.sync if s % 2 == 0 else nc.scalar
        eng.dma_start(out=qtt[:, :, js], in_=qtt_d[:, :, js])
        jc = slice(qnsl[s][0], qnsl[s][1])
        (nc.scalar if s % 2 == 0 else nc.sync).dma_start(
            out=qn8[:, jc], in_=qn8_d[:, jc])

    # ---- persistent phase-1 state ----------------------------------------
    # E^T[j,i] in fp16 (exp output; the ONLY reader of the S^T PSUM tiles --
    # a second PSUM reader would serialize against exp in the engine model)
    # and in fp8 (cast by DVE+Pool) for the DoubleRow U_A/Z matmuls.
    e16 = consts.tile([P, NJT, D], F16)
    e8 = consts.tile([P, NJT, D], F8)
    pre_e = consts.tile([P, 512], F16)
    # Two fp16 running-max accumulators (even/odd groups): consecutive chain
    # ops are independent, and fp16 gets the DVE 2x mode.  max(E) feeds b
    # directly (no second exp needed).
    m_e = consts.tile([P, 1024], F16)
    nc.vector.memset(m_e, 0.0)
    m_o = consts.tile([P, 1024], F16)
    nc.vector.memset(m_o, 0.0)

    ua_ps = [None] * NIC
    ua_ps[0] = uapool.tile([P, D], F32, tag="ua", name="ua0")
    z_ps = uapool.tile([P, D], F32, tag="ua", name="z_ps")
    nz = [0]

    def emit_z(g):
        for ic in range(NIC):
            nc.tensor.matmul(z_ps[:, ic:ic + 1],
                             lhsT=e8[:, 2 * g:2 * g + 2, ic * P:(ic + 1) * P],
                             rhs=ones8,
                             start=(nz[0] == 0), stop=(nz[0] == NG * NIC - 1),
                             perf_mode=DR, skip_group_check=True)
            nz[0] += 1

    # ---- phase 1: S^T -> exp -> (chain max, U_A for ic 0/1) ---------------
    # The e8-consuming U_A matmuls are emitted with a LAG of 2 groups: PE
    # executes in order, so placing them right after their group's S^T
    # matmuls would stall PE on that group's exp every iteration.
    LAG = 2

    def emit_consumers(g):
        for ic in (0,):
            nc.tensor.matmul(ua_ps[ic],
                             lhsT=e8[:, 2 * g:2 * g + 2, ic * P:(ic + 1) * P],
                             rhs=qn8[:, 2 * g:2 * g + 2, :],
                             start=(g == 0), stop=(g == NG - 1),
                             perf_mode=DR, skip_group_check=True)

    for g in range(NG):
        st = spool.tile([P, 1024], F32, tag="s", name=f"st{g}")
        for k in range(2):
            jt = 2 * g + k
            col = slice(k * 512, (k + 1) * 512)
            first = True
            for (lhs, rhs) in ((qt8, cw8), (qtr8, cw8), (qt8, cwr8)):
                for a in range(2):
                    nc.tensor.matmul(
                        st[:, col],
                        lhsT=lhs[:, 2 * a:2 * a + 2, jt * P:(jt + 1) * P],
                        rhs=rhs[:, 2 * a:2 * a + 2, :],
                        start=first, stop=(lhs is qt8 and rhs is cwr8
                                           and a == 1),
                        perf_mode=DR)
                    first = False
        # E^T (fp16) with the global shift; pair index == group index here
        nc.scalar.activation(out=e16[:, 2 * g:2 * g + 2, :], in_=st,
                             func=AF.Exp, bias=bias_t)
        # fp16 running max over groups (E domain -- feeds b directly).
        # Emitted BEFORE the DVE cast: both wait on the same exp, and the
        # in-order DVE queue would otherwise delay the critical final chain
        # by the cast's duration (casts feed lag-2 consumers with slack).
        macc = m_e if g % 2 == 0 else m_o
        nc.vector.tensor_tensor(out=macc, in0=e16[:, 2 * g:2 * g + 2, :],
                                in1=macc, op=mybir.AluOpType.max)
        # fp16->fp8 cast for the DR matmuls: DVE takes one jt, Pool the other
        nc.vector.tensor_copy(out=e8[:, 2 * g, :], in_=e16[:, 2 * g, :])
        nc.gpsimd.tensor_copy(out=e8[:, 2 * g + 1, :],
                              in_=e16[:, 2 * g + 1, :])
        if g == 6:
            nc.gpsimd.dma_start(out=c16, in_=c16_d)
        if g >= LAG:
            emit_consumers(g - LAG)
            emit_z(g - LAG)
        # partial max pre-folds once each accumulator is final
        if g == NG - 1:
            nc.vector.tensor_tensor(out=pre_e[:, 0:512],
                                    in0=m_e[:, 512:], in1=m_e[:, :512],
                                    op=mybir.AluOpType.max)
    for g in range(NG - LAG, NG):
        emit_consumers(g)
        emit_z(g)

    # ---- phase 2 ----------------------------------------------------------
    # Z finished during phase 1 (lagged); here: zinv, fold -> transpose ->
    # b16 -> h launch, then ALL remaining U_A under the h DMA round-trip.
    # The whole b -> h -> round-trip chain is emitted at high priority so
    # the scheduler never parks its DMAs behind the (slack-rich) G stores.
    with tc.high_priority():
        zinv = consts.tile([P, NIC], F32)
        nc.vector.reciprocal(out=zinv, in_=z_ps[:, 0:NIC])

        nc.vector.tensor_tensor(out=pre_e[:, 0:512], in0=m_o[:, 512:],
                                in1=pre_e[:, 0:512], op=mybir.AluOpType.max)
        mf = consts.tile([P, 512], F32)
        nc.vector.tensor_tensor(out=mf, in0=m_o[:, :512],
                                in1=pre_e[:, 0:512],
                                op=mybir.AluOpType.max)
        tp = uapool.tile([P, 512], F32, tag="ua", name="tp_m")
        for ic in range(NIC):
            nc.tensor.transpose(tp[:, ic * P:(ic + 1) * P],
                                mf[:, ic * P:(ic + 1) * P], ident32)
        emax = consts.tile([P, NIC], F32)
        nc.vector.tensor_reduce(out=emax,
                                in_=tp.rearrange("p (ic q) -> p ic q", q=P),
                                axis=mybir.AxisListType.X,
                                op=mybir.AluOpType.max)
        b16 = consts.tile([P, NIC], F16)
        nc.vector.tensor_tensor(out=b16, in0=emax, in1=zinv,
                                op=mybir.AluOpType.mult)

        # h partial; launch the DMA round-trip
        h_ps = uapool.tile([P, D], F32, tag="ua", name="h_ps")
        for ic in range(NIC):
            for dc in range(4):
                nc.tensor.matmul(h_ps[:, dc:dc + 1],
                                 lhsT=c16[:, ic, dc * P:(dc + 1) * P],
                                 rhs=b16[:, ic:ic + 1],
                                 start=(ic == 0 and dc == 0),
                                 stop=(ic == NIC - 1 and dc == 3),
                                 skip_group_check=True)
        h_sb = consts.tile([P, 4], F16)
        nc.vector.tensor_copy(out=h_sb, in_=h_ps[:, 0:4])
        hp_dram = dram.tile([D], F16)
        hs_dram = dram.tile([D], F16)
        hp_ap = hp_dram[:]
        nc.sync.dma_start(out=hp_ap.rearrange("(dc p) -> p dc", p=P),
                          in_=h_sb)
        if collective:
            nc.gpsimd.collective_compute(
                "AllReduce", mybir.AluOpType.add,
                replica_groups=[list(range(NCORES))],
                ins=[hp_dram.opt()], outs=[hs_dram.opt()],
            )
        else:
            nc.sync.dma_start(out=hs_dram[:], in_=hp_dram[:])
        hs_ap = hs_dram[:]
        h_bc = consts.tile([P, D], F16)
        nc.sync.dma_start(
            out=h_bc,
            in_=bass.AP(tensor=hs_ap.tensor, offset=hs_ap.offset,
                        ap=[[0, P], [1, D]]),
        )

    def emit_ua_phase2(ic):
        for pair in range(NPAIR):
            nc.tensor.matmul(ua_ps[ic],
                             lhsT=e8[:, 2 * pair:2 * pair + 2,
                                     ic * P:(ic + 1) * P],
                             rhs=qn8[:, 2 * pair:2 * pair + 2, :],
                             start=(pair == 0), stop=(pair == NPAIR - 1),
                             perf_mode=DR, skip_group_check=True)

    for ic in (1, 2, 3):
        ua_ps[ic] = uapool.tile([P, D], F32, tag="ua", name=f"ua{ic}")
        emit_ua_phase2(ic)

    # ---- G assembly (one fp16 staging tile) + 2 stores --------------------
    # gst[p, ic, :] holds G row ic*128+p.  Blocks: 0=c, 1=U_A, 2=c*U_A,
    # 3=c*h.  cu/ch are all-fp16 DVE products (2x mode); block 3 waits only
    # on the h broadcast.
    gst = gpool.tile([P, NIC, 4 * D], F16)
    nc.vector.tensor_copy(out=gst[:, :, 0:D], in_=c16)
    for ic in range(NIC):
        nc.scalar.activation(out=gst[:, ic, D:2 * D], in_=ua_ps[ic],
                             func=AF.Copy, scale=zinv[:, ic:ic + 1])
        nc.vector.tensor_tensor(out=gst[:, ic, 2 * D:3 * D],
                                in0=c16[:, ic, :], in1=gst[:, ic, D:2 * D],
                                op=mybir.AluOpType.mult)
        nc.scalar.dma_start(out=g_d[ic * P:(ic + 1) * P, 0:3 * D],
                             in_=gst[:, ic, 0:3 * D])
    with tc.high_priority():
        h_bc4 = bass.AP(tensor=h_bc.tensor, offset=h_bc.offset,
                        ap=[h_bc.ap[0], [0, NIC], h_bc.ap[1]])
        nc.vector.tensor_tensor(out=gst[:, :, 3 * D:4 * D], in0=c16,
                                in1=h_bc4, op=mybir.AluOpType.mult)
        nc.sync.dma_start(
            out=g_d.rearrange("(ic p) c -> p ic c", p=P)[:, :, 3 * D:4 * D],
            in_=gst[:, :, 3 * D:4 * D])

    ctx.close()


# ---------------------------------------------------------------------------


def _prep_inputs(x, w):
    """Host-side quantization + layout. Returns per-core in_maps."""
    context = np.ascontiguousarray(x[0, 0]).astype(np.float32)   # (T, D)
    question = np.ascontiguousarray(x[1, 0]).astype(np.float32)  # (T, D)
    w = np.asarray(w, dtype=np.float32)
    w2 = w[D:2 * D]
    w3 = w[2 * D:3 * D]

    # question.T in [p, dc, j] layout, fp8 + fp8 residual
    qT = question.T.reshape(4, P, T)                  # [dc, p, j]
    qT = np.ascontiguousarray(qT.transpose(1, 0, 2))  # [p, dc, j]
    qt8 = qT.astype(NP8)
    qtr8 = (qT - qt8.astype(np.float32)).astype(NP8)
    qtt = np.concatenate([qt8, qtr8], axis=1)         # [p, 8, j]

    # question natural in [p, jc, d] layout, fp8
    qn = question.reshape(NJT, P, D)                  # [jc, p, d]
    qn8 = np.ascontiguousarray(qn.transpose(1, 0, 2)).astype(NP8)

    cw_full = context * w3[None, :] + w2[None, :]     # (T, D)

    in_maps = []
    for core in range(NCORES):
        rows = slice(core * TL, (core + 1) * TL)
        cw = cw_full[rows]                            # (TL, D)
        cwT = cw.T.reshape(4, P, TL)                  # [dc, p, i]
        cwT = np.ascontiguousarray(cwT.transpose(1, 0, 2))
        cw8 = cwT.astype(NP8)
        cwr8 = (cwT - cw8.astype(np.float32)).astype(NP8)
        cwp = np.concatenate([cw8, cwr8], axis=1)     # [p, 8, i]
        cn = context[rows].reshape(NIC, P, D)         # [ic, p, d]
        c16 = np.ascontiguousarray(cn.transpose(1, 0, 2)).astype(np.float16)
        in_maps.append({
            "qtt": qtt, "qn8": qn8, "cwp": cwp, "c16": c16,
        })
    return in_maps


_NC_CACHE = {}


def _get_nc():
    if "nc" not in _NC_CACHE:
        _NC_CACHE["nc"] = build_kernel()
    return _NC_CACHE["nc"]


def kernel(x: np.ndarray, kernel: np.ndarray) -> np.ndarray:
    nc = _get_nc()
    in_maps = _prep_inputs(x, kernel)
    res = run_bass_kernel_spmd(nc, in_maps, core_ids=list(range(NCORES)))
    g = np.concatenate([res.results[core]["g"] for core in range(NCORES)],
                       axis=0)
    return g.astype(np.float32)
